# revision 61
# baseline (speedup 1.0000x reference)
"""Trainium2 Bass kernel for nn_Net_25847113187867 (dense_cnn).

The reference slides W = 16384 stride-1 windows over x (1,2,L), runs
conv(s5)/conv(s3)/conv(s2) + 3-layer MLP + hidden-size-1 Elman RNN per
window, twice (second pass with x channel 0 negated), and returns the
antisymmetrized scan outputs (y - y_)/2.

Restructure (v2):
  * Window conv stack == dilated convs over the full sequence; fc3+RNN
    input row folded into one 80->1 vector on the host.
  * Channel-0 negation == negated conv1 weights; passes share x.
  * Sources stored stacked [src[p]; src[p+d]] across 128 partitions so
    each 64->64 dilated conv is 3 full-contraction matmuls per block.
    Stacked lower halves evacuated from PSUM (DVE/ACT split); shifted
    upper halves produced by SBUF->SBUF DVE copies.
  * conv2/conv3, fc1 and fc2 run in bf16 (fp32 PSUM accumulate,
    ~6e-3 rel err contribution vs the 2e-2 gate); conv1 and the final
    80->1 fold stay f32r.  bf16 legalizes PE-array col tiling (fp32
    modes require dst partition 0), so pass A and pass B run
    CONCURRENTLY in separate array halves wherever M<=64:
      - conv2/conv3: A -> psum[0:64] (col grp 0-1), B -> psum[64:128].
      - fc1 c=2 chunk (M=64): col-tiled pair + quad-tiled K=64 tap.
      - fc1/fc2 K=64 leftover taps (F1S/F2S): row-tiled pairs, B's
        weights+rhs at SBUF partitions 64:128 (B's rhs uses the
        shifted-copy identity U[64:128,c] = U[0:64,c+30]).
  * VV (fc3/RNN fold) matmuls batched after all fc2 groups (no PE
    head-of-line stall on the Y2 relu evacuation).
  * tanh scan: A and B chunk rows merged in one [128, 77] tile set, one
    DEER chain for both passes.
  * One f32 warmup matmul on zeros pre-warms the PE HAM clock gate
    during the input DMA window; input DMAs split across the two HW
    DGE queues (sync + scalar), small tensors packed.
  * 8 cores split outputs into 2048-position slices (overlapping input
    halos, no collectives).  All matmul free dims >= 256 so f32r
    streams at full PE rate.
"""

import numpy as np

L = 16684
W = 16384
P = 2048            # output positions per core
CH = 33             # scan chunk length (output steps per chunk row)
KW = 44             # per-chunk warmup halo steps (|whh|^44 * 0.33 ~ 3e-6)
SC = KW + CH        # 77 scan columns per chunk row
HALO = KW           # 44: left halo of xp positions per core
NY = 62 * CH + SC + 1  # 2124 xp positions per core: [s-44, s+2080)
NC3 = NY + 180      # 2304 c3 positions per core
NC2 = NC3 + 76      # 2380
NC1 = NC2 + 26      # 2406
NX = NC1 + 6        # 2412
SCAN_ITERS = 2

NY_GROUPS = [(0, 1024), (1024, 844), (1868, 256)]  # all blocks >= 256


def _groups(n, g=1024):
    out, o = [], 0
    while o < n:
        out.append((o, min(g, n - o)))
        o += g
    return out


def _subs(gw):
    s = [(0, min(512, gw))]
    if gw > 512:
        s.append((512, gw - 512))
    return s


def _build_program(a_const, c0_const):
    import concourse.bass as bass
    import concourse.mybir as mybir
    import concourse.tile as tile
    from concourse import bacc
    from contextlib import ExitStack

    dt = mybir.dt
    f32 = dt.float32
    f32r = dt.float32r
    bf16 = dt.bfloat16
    AF = mybir.ActivationFunctionType
    OP = mybir.AluOpType

    nc = bacc.Bacc("TRN2", target_bir_lowering=False, debug=False,
                   num_devices=8)

    # xww = xw | w1 (both [10,*] f32r); wb = f1p|f1sd|w2|w3|f2|f2sd (bf16)
    xww_d = nc.dram_tensor("xww", [10, NX + 128], f32r, kind="ExternalInput")
    wb_d = nc.dram_tensor("wb", [128, 1904], bf16, kind="ExternalInput")
    vv_d = nc.dram_tensor("vv", [80, 1], f32r, kind="ExternalInput")
    bias_d = nc.dram_tensor("bias", [128, 7], f32, kind="ExternalInput")
    mask_d = nc.dram_tensor("mask", [128, SC], f32, kind="ExternalInput")
    y_d = nc.dram_tensor("y", [1, P], f32, kind="ExternalOutput")

    a = float(a_const)
    c0 = float(c0_const)

    with ExitStack() as ctx:
        tc = ctx.enter_context(tile.TileContext(nc))
        wp = ctx.enter_context(tc.tile_pool(name="weights", bufs=1))
        sp = ctx.enter_context(tc.tile_pool(name="acts", bufs=1))
        pp = ctx.enter_context(tc.tile_pool(name="ps", bufs=4, space="PSUM"))

        # ---- weight loads: few big packed DMAs on the two DGE queues ----
        def load(eng, dram, shape, name, dtype):
            t = wp.tile(shape, dtype, name=name, tag=name)
            eng.dma_start(t[:], dram.ap())
            return t

        XWW = load(nc.sync, xww_d, [10, NX + 128], "xww", f32r)
        BIAS = load(nc.scalar, bias_d, [128, 7], "bias", f32)
        WB = load(nc.scalar, wb_d, [128, 1904], "wb", bf16)
        MASK = load(nc.sync, mask_d, [128, SC], "mask", f32)
        VV = load(nc.scalar, vv_d, [80, 1], "vv", f32r)

        class _CV:
            """Column view into a packed weight tile."""

            def __init__(self, t, base, ncols):
                self.t, self.base, self.n = t, base, ncols

            def __getitem__(self, idx):
                r, c = idx
                lo = self.base + (c.start if c.start is not None else 0)
                hi = self.base + (c.stop if c.stop is not None else self.n)
                return self.t[r, lo:hi]

        XW = _CV(XWW, 0, NX)
        W1 = _CV(XWW, NX, 128)
        F1P = _CV(WB, 0, 960)
        F1SD = _CV(WB, 960, 320)
        W2 = _CV(WB, 1280, 192)
        W3 = _CV(WB, 1472, 192)
        F2 = _CV(WB, 1664, 160)
        F2SD = _CV(WB, 1824, 80)

        # ---- activation tiles (stacked [lo; lo-shifted] layout) ----
        SA = sp.tile([128, NC1], bf16, name="SA", tag="SA")
        SB = sp.tile([128, NC1], bf16, name="SB", tag="SB")
        TA = sp.tile([128, NC2], bf16, name="TA", tag="TA")
        TB = sp.tile([128, NC2], bf16, name="TB", tag="TB")
        UA = sp.tile([128, NC3], bf16, name="UA", tag="UA")
        UB = sp.tile([128, NC3], bf16, name="UB", tag="UB")
        Y1 = {(p, c): sp.tile([128, NY], bf16, name=f"Y1{p}{c}",
                              tag=f"Y1{p}{c}")
              for p in "AB" for c in (0, 1)}
        Y12 = sp.tile([128, NY], bf16, name="Y12", tag="Y12")
        Y2A = sp.tile([80, NY], f32r, name="Y2A", tag="Y2A")
        Y2B = sp.tile([80, NY], f32r, name="Y2B", tag="Y2B")
        XPRA = sp.tile([1, NY], f32, name="XPRA", tag="XPRA")
        XPRB = sp.tile([1, NY], f32, name="XPRB", tag="XPRB")

        def stile(nm):
            return sp.tile([128, SC], f32, name=nm, tag=nm)

        SCT, ZT, FT, GT, DT, BT, ET, HT, H2T = (
            stile(n) for n in ("SCT", "Z", "F", "G", "DD", "BB", "E",
                               "H", "H2"))
        D = sp.tile([64, CH], f32, name="D", tag="D")
        Z0 = sp.tile([128, 512], bf16, name="Z0", tag="Z0")

        # ---- PE warmup + clock-keeper matmuls.  The HAM clock gate only
        # credits windows where the full contraction dim streams (K=128;
        # K=10 c1 and K=64 f32 warmups never un-throttled it) AND the
        # data toggles (all-zero and constant-pattern operands never
        # registered; iota data did).  A run of K=128 bf16 matmuls on
        # iota data warms the gate during the input-DMA window, and a
        # few more dependency-free "keeper" matmuls dropped between
        # stages keep the PE busy across the evacuation-wait gaps so
        # the gate never re-throttles mid-kernel.
        for par, val in ((0, 1.0), (1, -7.3)):
            nc.vector.memset(
                bass.AP(tensor=Z0.tensor, offset=Z0.offset + par,
                        ap=[[512, 128], [2, 256]]), val)
        nc.gpsimd.memset(SCT[0:1, :], 0.0)
        nc.gpsimd.memset(SCT[64:65, :], 0.0)
        nc.gpsimd.memset(BT[:, 0:1], 0.0)
        psW = pp.tile([128, 1024], f32, name="ps", tag="ps")

        def keeper(n):
            for _ in range(n):
                nc.tensor.matmul(psW[:, 0:512], Z0[:, 0:128], Z0[:, 0:512],
                                 start=True, stop=True)

        keeper(6)

        # ================= c1 (both passes in one M=128 matmul) =========
        # 512-col groups so evacuations pipeline with the matmuls.
        def shift_copy(eng, DST, goff, gw, shift):
            if goff == 0:
                eng.tensor_copy(DST[64:128, 0:gw - shift],
                                DST[0:64, shift:gw])
            else:
                eng.tensor_copy(DST[64:128, goff - shift:goff + gw - shift],
                                DST[0:64, goff:goff + gw])

        for goff, gw in _groups(NC1, 512):
            ps = pp.tile([128, 1024], f32, name="ps", tag="ps")
            nc.tensor.matmul(ps[:, 0:gw], W1[:, :], XW[:, goff:goff + gw],
                             start=True, stop=True)
            nc.vector.tensor_scalar(SA[0:64, goff:goff + gw], ps[0:64, :gw],
                                    BIAS[0:64, 0:1], 0.0, OP.add, OP.max)
            nc.scalar.activation(SB[0:64, goff:goff + gw], ps[64:128, :gw],
                                 AF.Relu, bias=BIAS[64:128, 0:1])
            shift_copy(nc.vector, SA, goff, gw, 5)
            shift_copy(nc.vector, SB, goff, gw, 5)

        # ============ c2/c3: col-tiled A||B dilated convs (bf16) ========
        # Stacked [lo; lo<<dil] sources so each conv is 3 K=128 matmuls
        # (K=128 is also what keeps the HAM clock gate fed — K<=64 work
        # does not register and the PE clock halves).  Keepers pad the
        # per-group evacuation wait so the gate never sees an idle
        # window.
        def conv_stage(SRCA, SRCB, DSTA, DSTB, Wt, bcol, n_out, dil, shift):
            for gi, (goff, gw) in enumerate(_groups(n_out)):
                ps = pp.tile([128, 1024], f32, name="ps", tag="ps")
                for bo, nb in _subs(gw):
                    o = goff + bo
                    for t in range(3):
                        for SRC, rows in ((SRCA, slice(0, 64)),
                                          (SRCB, slice(64, 128))):
                            nc.tensor.matmul(
                                ps[rows, bo:bo + nb],
                                Wt[:, 64 * t:64 * t + 64],
                                SRC[:, o + 2 * dil * t:o + 2 * dil * t + nb],
                                start=(t == 0), stop=(t == 2),
                                skip_group_check=True)
                nc.vector.tensor_scalar(DSTA[0:64, goff:goff + gw],
                                        ps[0:64, :gw],
                                        BIAS[0:64, bcol:bcol + 1], 0.0,
                                        OP.add, OP.max)
                nc.scalar.activation(DSTB[0:64, goff:goff + gw],
                                     ps[64:128, :gw], AF.Relu,
                                     bias=BIAS[64:128, bcol:bcol + 1])
                shift_copy(nc.vector, DSTA, goff, gw, shift)
                shift_copy(nc.vector, DSTB, goff, gw, shift)

        conv_stage(SA, SB, TA, TB, W2, 1, NC2, 5, 15)
        conv_stage(TA, TB, UA, UB, W3, 2, NC3, 15, 30)

        # ======================= fc1 (bf16) =============================
        for goff, gw in NY_GROUPS:
            for c in range(2):      # M=128 chunks; F1S pair row-tiled
                psA = pp.tile([128, 1024], f32, name="ps", tag="ps")
                psB = pp.tile([128, 1024], f32, name="ps", tag="ps")
                for bo, nb in _subs(gw):
                    o = goff + bo
                    for ps_, U in ((psA, UA), (psB, UB)):
                        for p in range(3):
                            nc.tensor.matmul(
                                ps_[:, bo:bo + nb],
                                F1P[:, 320 * p + 128 * c:320 * p + 128 * c + 128],
                                U[:, o + 60 * p:o + 60 * p + nb],
                                start=(p == 0), stop=False)
                    nc.tensor.matmul(
                        psA[:, bo:bo + nb], F1SD[0:64, 128 * c:128 * c + 128],
                        UA[0:64, o + 180:o + 180 + nb],
                        start=False, stop=True)
                    nc.tensor.matmul(
                        psB[:, bo:bo + nb], F1SD[64:128, 128 * c:128 * c + 128],
                        UB[64:128, o + 150:o + 150 + nb],
                        start=False, stop=True)
                nc.vector.tensor_scalar(Y1[("A", c)][:, goff:goff + gw],
                                        psA[:, :gw], BIAS[:, 3 + c:4 + c],
                                        0.0, OP.add, OP.max)
                nc.scalar.activation(Y1[("B", c)][:, goff:goff + gw],
                                     psB[:, :gw], AF.Relu,
                                     bias=BIAS[:, 3 + c:4 + c])
            # c=2 chunk (M=64): col-tiled A||B, F1S quad-tiled
            psC = pp.tile([128, 1024], f32, name="ps", tag="ps")
            for bo, nb in _subs(gw):
                o = goff + bo
                for p in range(3):
                    for U, rows in ((UA, slice(0, 64)), (UB, slice(64, 128))):
                        nc.tensor.matmul(
                            psC[rows, bo:bo + nb],
                            F1P[:, 320 * p + 256:320 * p + 320],
                            U[:, o + 60 * p:o + 60 * p + nb],
                            start=(p == 0), stop=False,
                            skip_group_check=True)
                nc.tensor.matmul(psC[0:64, bo:bo + nb],
                                 F1SD[0:64, 256:320],
                                 UA[0:64, o + 180:o + 180 + nb],
                                 start=False, stop=True,
                                 skip_group_check=True)
                nc.tensor.matmul(psC[64:128, bo:bo + nb],
                                 F1SD[64:128, 256:320],
                                 UB[64:128, o + 150:o + 150 + nb],
                                 start=False, stop=True,
                                 skip_group_check=True)
            nc.vector.tensor_scalar(Y12[:, goff:goff + gw], psC[:, :gw],
                                    BIAS[:, 5:6], 0.0, OP.add, OP.max)

        # ======================= fc2 (bf16, Y2 only) ====================
        for goff, gw in NY_GROUPS:
            psA = pp.tile([128, 1024], f32, name="ps", tag="ps")
            psB = pp.tile([128, 1024], f32, name="ps", tag="ps")
            for bo, nb in _subs(gw):
                o = goff + bo
                for ps_, k0, k1 in ((psA, Y1[("A", 0)], Y1[("A", 1)]),
                                    (psB, Y1[("B", 0)], Y1[("B", 1)])):
                    nc.tensor.matmul(ps_[:80, bo:bo + nb], F2[:, 0:80],
                                     k0[:, o:o + nb], start=True, stop=False)
                    nc.tensor.matmul(ps_[:80, bo:bo + nb], F2[:, 80:160],
                                     k1[:, o:o + nb], start=False, stop=False)
                nc.tensor.matmul(psA[:80, bo:bo + nb], F2SD[0:64, 0:80],
                                 Y12[0:64, o:o + nb], start=False, stop=True)
                nc.tensor.matmul(psB[:80, bo:bo + nb], F2SD[64:128, 0:80],
                                 Y12[64:128, o:o + nb], start=False, stop=True)
            nc.vector.tensor_scalar(Y2A[:, goff:goff + gw], psA[:80, :gw],
                                    BIAS[0:80, 6:7], 0.0, OP.add, OP.max)
            nc.scalar.activation(Y2B[:, goff:goff + gw], psB[:80, :gw],
                                 AF.Relu, bias=BIAS[0:80, 6:7])

        # ============== VV batch (f32r, M=1, dst partition 0) ===========
        # Each NY group's xp chunk rows gather into SCT as soon as that
        # group's XPR columns land (rows r with 33(r-1)+77 <= group end),
        # so only the last small gather trails the final VV matmul.
        # SCT rows 1:64 = pass-A chunk rows 1..63, rows 65:128 = pass B.
        # Row r covers xp positions s-44+33*(r-1)+j, j in [0,77).
        # rows r with 33(r-1)+77 <= group end: 1024 -> r<=29, 1868 -> r<=55
        GATHER_ROWS = [(1, 30), (30, 56), (56, 64)]
        for gi, (goff, gw) in enumerate(NY_GROUPS):
            psA3 = pp.tile([128, 1024], f32, name="ps", tag="ps")
            psB3 = pp.tile([128, 1024], f32, name="ps", tag="ps")
            for bo, nb in _subs(gw):
                o = goff + bo
                nc.tensor.matmul(psA3[0:1, bo:bo + nb], VV[:, :],
                                 Y2A[:, o:o + nb], start=True, stop=True)
                nc.tensor.matmul(psB3[0:1, bo:bo + nb], VV[:, :],
                                 Y2B[:, o:o + nb], start=True, stop=True)
            nc.vector.tensor_scalar(XPRA[0:1, goff:goff + gw], psA3[0:1, :gw],
                                    c0, None, OP.add)
            nc.scalar.activation(XPRB[0:1, goff:goff + gw], psB3[0:1, :gw],
                                 AF.Copy, bias=c0)
            r0, r1 = GATHER_ROWS[gi]
            engs = [(nc.sync, nc.scalar), (nc.gpsimd, nc.sync),
                    (nc.scalar, nc.gpsimd)][gi]
            for XPR, rbase, eng in ((XPRA, 0, engs[0]), (XPRB, 64, engs[1])):
                src = bass.AP(tensor=XPR.tensor,
                              offset=XPR.offset + CH * (r0 - 1),
                              ap=[[NY, 1], [CH, r1 - r0], [1, SC]])
                eng.dma_start(SCT[rbase + r0:rbase + r1, :], src)
        nc.vector.tensor_tensor(SCT[:, :], SCT[:, :], MASK[:, :], OP.mult)
        nc.vector.tensor_copy(ZT[:, 0:1], SCT[:, 0:1])
        nc.scalar.activation(HT[:, :], SCT[:, :], AF.Tanh)
        cur, nxt = HT, H2T
        for it in range(SCAN_ITERS):
            nc.vector.scalar_tensor_tensor(ZT[:, 1:SC], cur[:, 0:SC - 1],
                                           a, SCT[:, 1:SC], OP.mult, OP.add)
            nc.scalar.activation(FT[:, :], ZT[:, :], AF.Tanh)
            nc.vector.tensor_tensor(GT[:, :], FT[:, :], FT[:, :], OP.mult)
            nc.vector.tensor_scalar(GT[:, :], GT[:, :], -a, a, OP.mult,
                                    OP.add)
            nc.vector.tensor_tensor(DT[:, :], FT[:, :], cur[:, :],
                                    OP.subtract)
            nc.vector.tensor_tensor(BT[:, 1:SC], GT[:, 1:SC],
                                    DT[:, 0:SC - 1], OP.mult)
            nc.vector.tensor_tensor_scan(ET[:, :], GT[:, :], BT[:, :],
                                         0.0, OP.mult, OP.add)
            nc.vector.tensor_tensor(nxt[:, :], FT[:, :], ET[:, :], OP.add)
            cur, nxt = nxt, cur

        # y[p] = (hA[p] - hB[p]) / 2: binary DVE ops need equal src base
        # partitions, so bounce B's half down via a unary copy first.
        nc.vector.tensor_copy(nxt[0:64, KW:SC], cur[64:128, KW:SC])
        nc.vector.tensor_tensor(D[:, :], cur[0:64, KW:SC],
                                nxt[0:64, KW:SC], OP.subtract)
        nc.vector.tensor_scalar(D[:, :], D[:, :], 0.5, None, OP.mult)
        nc.sync.dma_start(
            y_d.ap()[0, 0:62 * CH].rearrange("(r c) -> r c", c=CH),
            D[1:63, :])
        nc.sync.dma_start(y_d.ap()[0:1, 62 * CH:P], D[63:64, 0:P - 62 * CH])

    nc.compile()
    return nc


def _prep_inputs(inputs):
    """Host-side packing: per-core input dicts."""
    import ml_dtypes
    bfd = ml_dtypes.bfloat16

    x0 = np.asarray(inputs["x0"], np.float32)[0]
    w1 = np.asarray(inputs["conv1_w"], np.float32)
    b1 = np.asarray(inputs["conv1_b"], np.float32)
    w2 = np.asarray(inputs["conv2_w"], np.float32)
    b2 = np.asarray(inputs["conv2_b"], np.float32)
    w3 = np.asarray(inputs["conv3_w"], np.float32)
    b3 = np.asarray(inputs["conv3_b"], np.float32)
    f1w = np.asarray(inputs["fc1_w"], np.float32)
    f1b = np.asarray(inputs["fc1_b"], np.float32)
    f2w = np.asarray(inputs["fc2_w"], np.float32)
    f2b = np.asarray(inputs["fc2_b"], np.float32)
    f3w = np.asarray(inputs["fc3_w"], np.float32)
    f3b = np.asarray(inputs["fc3_b"], np.float32)
    wih = np.asarray(inputs["rnn_wih"], np.float32)
    whh = np.asarray(inputs["rnn_whh"], np.float32)
    bih = np.asarray(inputs["rnn_bih"], np.float32)
    bhh = np.asarray(inputs["rnn_bhh"], np.float32)

    a = float(whh[0, 0])
    v = (wih @ f3w)[0]
    c0 = float((wih @ f3b + bih + bhh)[0])

    W1 = np.zeros((10, 128), np.float32)
    for c in range(2):
        for k in range(5):
            W1[c * 5 + k, 0:64] = w1[:, c, k]
            W1[c * 5 + k, 64:128] = w1[:, c, k] * (-1.0 if c == 0 else 1.0)

    def pack_pairs(w):  # (64,64,6) -> [128, 192]
        out = np.zeros((128, 192), np.float32)
        for t in range(3):
            out[0:64, 64 * t:64 * t + 64] = w[:, :, 2 * t].T
            out[64:128, 64 * t:64 * t + 64] = w[:, :, 2 * t + 1].T
        return out

    W2 = pack_pairs(w2)
    W3 = pack_pairs(w3)

    f1r = f1w.reshape(320, 64, 7)  # flat index = ch*7 + m
    F1P = np.zeros((128, 960), np.float32)
    for p in range(3):
        F1P[0:64, 320 * p:320 * p + 320] = f1r[:, :, 2 * p].T
        F1P[64:128, 320 * p:320 * p + 320] = f1r[:, :, 2 * p + 1].T
    F1SD = np.zeros((128, 320), np.float32)
    F1SD[0:64] = f1r[:, :, 6].T
    F1SD[64:128] = f1r[:, :, 6].T

    F2 = np.zeros((128, 160), np.float32)
    F2[:, 0:80] = f2w[:, 0:128].T
    F2[:, 80:160] = f2w[:, 128:256].T
    F2SD = np.zeros((128, 80), np.float32)
    F2SD[0:64] = f2w[:, 256:320].T
    F2SD[64:128] = f2w[:, 256:320].T

    BIAS = np.zeros((128, 7), np.float32)
    BIAS[:, 0] = np.concatenate([b1, b1])
    BIAS[:, 1] = np.concatenate([b2, b2])
    BIAS[:, 2] = np.concatenate([b3, b3])
    BIAS[:, 3] = f1b[0:128]
    BIAS[:, 4] = f1b[128:256]
    BIAS[0:64, 5] = f1b[256:320]
    BIAS[64:128, 5] = f1b[256:320]
    BIAS[0:80, 6] = f2b

    lpad = HALO
    rpad = (7 * P - HALO + NX + 8) - L
    xpad = np.zeros((2, lpad + L + max(rpad, 0)), np.float32)
    xpad[:, lpad:lpad + L] = x0

    WB = np.concatenate([F1P, F1SD, W2, W3, F2, F2SD],
                        axis=1).astype(bfd)  # [128, 1904]
    shared = dict(wb=WB, vv=v.reshape(80, 1), bias=BIAS)

    in_maps = []
    for core in range(8):
        s = P * core
        base = lpad + s - HALO
        xww = np.zeros((10, NX + 128), np.float32)
        for c in range(2):
            for k in range(5):
                xww[c * 5 + k, 0:NX] = xpad[c, base + k:base + k + NX]
        xww[:, NX:NX + 128] = W1
        # scan mask rows 0:64 = pass A chunk rows, 64:128 = pass B (same):
        # row r>=1 col j is position s - 44 + 33*(r-1) + j; zero where
        # position < 0 (core 0 only)
        m64 = np.ones((64, SC), np.float32)
        if core == 0:
            for rr in range(1, 64):
                for j in range(SC):
                    if s - HALO + CH * (rr - 1) + j < 0:
                        m64[rr, j] = 0.0
        mask = np.concatenate([m64, m64], axis=0)
        m = dict(shared)
        m["xww"] = xww
        m["mask"] = mask
        in_maps.append(m)
    return in_maps, a, c0


LAST_RESULT = None


def kernel(**inputs) -> np.ndarray:
    global LAST_RESULT
    from concourse import bass_utils

    in_maps, a, c0 = _prep_inputs(inputs)
    nc = _build_program(a, c0)
    res = bass_utils.run_bass_kernel_spmd(nc, in_maps, core_ids=list(range(8)))
    LAST_RESULT = res
    out = np.empty((1, W), np.float32)
    for core in range(8):
        out[0, P * core:P * core + P] = res.results[core]["y"][0]
    return out


# revision 64
# speedup vs baseline: 1.0031x; 1.0031x over previous
"""Trainium2 Bass kernel for nn_Net_25847113187867 (dense_cnn).

The reference slides W = 16384 stride-1 windows over x (1,2,L), runs
conv(s5)/conv(s3)/conv(s2) + 3-layer MLP + hidden-size-1 Elman RNN per
window, twice (second pass with x channel 0 negated), and returns the
antisymmetrized scan outputs (y - y_)/2.

Restructure (v2):
  * Window conv stack == dilated convs over the full sequence; fc3+RNN
    input row folded into one 80->1 vector on the host.
  * Channel-0 negation == negated conv1 weights; passes share x.
  * Sources stored stacked [src[p]; src[p+d]] across 128 partitions so
    each 64->64 dilated conv is 3 full-contraction matmuls per block.
    Stacked lower halves evacuated from PSUM (DVE/ACT split); shifted
    upper halves produced by SBUF->SBUF DVE copies.
  * conv2/conv3, fc1 and fc2 run in bf16 (fp32 PSUM accumulate,
    ~6e-3 rel err contribution vs the 2e-2 gate); conv1 and the final
    80->1 fold stay f32r.  bf16 legalizes PE-array col tiling (fp32
    modes require dst partition 0), so pass A and pass B run
    CONCURRENTLY in separate array halves wherever M<=64:
      - conv2/conv3: A -> psum[0:64] (col grp 0-1), B -> psum[64:128].
      - fc1 c=2 chunk (M=64): col-tiled pair + quad-tiled K=64 tap.
      - fc1/fc2 K=64 leftover taps (F1S/F2S): row-tiled pairs, B's
        weights+rhs at SBUF partitions 64:128 (B's rhs uses the
        shifted-copy identity U[64:128,c] = U[0:64,c+30]).
  * VV (fc3/RNN fold) matmuls batched after all fc2 groups (no PE
    head-of-line stall on the Y2 relu evacuation).
  * tanh scan: A and B chunk rows merged in one [128, 77] tile set, one
    DEER chain for both passes.
  * One f32 warmup matmul on zeros pre-warms the PE HAM clock gate
    during the input DMA window; input DMAs split across the two HW
    DGE queues (sync + scalar), small tensors packed.
  * 8 cores split outputs into 2048-position slices (overlapping input
    halos, no collectives).  All matmul free dims >= 256 so f32r
    streams at full PE rate.
"""

import numpy as np

L = 16684
W = 16384
P = 2048            # output positions per core
CH = 33             # scan chunk length (output steps per chunk row)
KW = 44             # per-chunk warmup halo steps (|whh|^44 * 0.33 ~ 3e-6)
SC = KW + CH        # 77 scan columns per chunk row
HALO = KW           # 44: left halo of xp positions per core
NY = 62 * CH + SC + 1  # 2124 xp positions per core: [s-44, s+2080)
NC3 = NY + 180      # 2304 c3 positions per core
NC2 = NC3 + 76      # 2380
NC1 = NC2 + 26      # 2406
NX = NC1 + 6        # 2412
SCAN_ITERS = 2

NY_GROUPS = [(0, 1024), (1024, 844), (1868, 256)]  # all blocks >= 256


def _groups(n, g=1024):
    out, o = [], 0
    while o < n:
        out.append((o, min(g, n - o)))
        o += g
    return out


def _subs(gw):
    s = [(0, min(512, gw))]
    if gw > 512:
        s.append((512, gw - 512))
    return s


def _build_program(a_const, c0_const):
    import concourse.bass as bass
    import concourse.mybir as mybir
    import concourse.tile as tile
    from concourse import bacc
    from contextlib import ExitStack

    dt = mybir.dt
    f32 = dt.float32
    f32r = dt.float32r
    bf16 = dt.bfloat16
    AF = mybir.ActivationFunctionType
    OP = mybir.AluOpType

    nc = bacc.Bacc("TRN2", target_bir_lowering=False, debug=False,
                   num_devices=8)

    # xww = xw | w1 (both [10,*] f32r); wb = f1p|f1sd|w2|w3|f2|f2sd (bf16)
    xww_d = nc.dram_tensor("xww", [10, NX + 128], f32r, kind="ExternalInput")
    wb_d = nc.dram_tensor("wb", [128, 1904], bf16, kind="ExternalInput")
    vv_d = nc.dram_tensor("vv", [80, 1], f32r, kind="ExternalInput")
    bias_d = nc.dram_tensor("bias", [128, 7], f32, kind="ExternalInput")
    mask_d = nc.dram_tensor("mask", [128, SC], f32, kind="ExternalInput")
    y_d = nc.dram_tensor("y", [1, P], f32, kind="ExternalOutput")

    a = float(a_const)
    c0 = float(c0_const)

    with ExitStack() as ctx:
        tc = ctx.enter_context(tile.TileContext(nc))
        wp = ctx.enter_context(tc.tile_pool(name="weights", bufs=1))
        sp = ctx.enter_context(tc.tile_pool(name="acts", bufs=1))
        pp = ctx.enter_context(tc.tile_pool(name="ps", bufs=4, space="PSUM"))

        # ---- weight loads: few big packed DMAs on the two DGE queues ----
        def load(eng, dram, shape, name, dtype):
            t = wp.tile(shape, dtype, name=name, tag=name)
            eng.dma_start(t[:], dram.ap())
            return t

        XWW = load(nc.sync, xww_d, [10, NX + 128], "xww", f32r)
        BIAS = load(nc.scalar, bias_d, [128, 7], "bias", f32)
        WB = load(nc.scalar, wb_d, [128, 1904], "wb", bf16)
        MASK = load(nc.sync, mask_d, [128, SC], "mask", f32)
        VV = load(nc.scalar, vv_d, [80, 1], "vv", f32r)

        class _CV:
            """Column view into a packed weight tile."""

            def __init__(self, t, base, ncols):
                self.t, self.base, self.n = t, base, ncols

            def __getitem__(self, idx):
                r, c = idx
                lo = self.base + (c.start if c.start is not None else 0)
                hi = self.base + (c.stop if c.stop is not None else self.n)
                return self.t[r, lo:hi]

        XW = _CV(XWW, 0, NX)
        W1 = _CV(XWW, NX, 128)
        F1P = _CV(WB, 0, 960)
        F1SD = _CV(WB, 960, 320)
        W2 = _CV(WB, 1280, 192)
        W3 = _CV(WB, 1472, 192)
        F2 = _CV(WB, 1664, 160)
        F2SD = _CV(WB, 1824, 80)

        # ---- activation tiles (stacked [lo; lo-shifted] layout) ----
        SA = sp.tile([128, NC1], bf16, name="SA", tag="SA")
        SB = sp.tile([128, NC1], bf16, name="SB", tag="SB")
        TA = sp.tile([128, NC2], bf16, name="TA", tag="TA")
        TB = sp.tile([128, NC2], bf16, name="TB", tag="TB")
        UA = sp.tile([128, NC3], bf16, name="UA", tag="UA")
        UB = sp.tile([128, NC3], bf16, name="UB", tag="UB")
        Y1 = {(p, c): sp.tile([128, NY], bf16, name=f"Y1{p}{c}",
                              tag=f"Y1{p}{c}")
              for p in "AB" for c in (0, 1)}
        Y12 = sp.tile([128, NY], bf16, name="Y12", tag="Y12")
        Y2A = sp.tile([80, NY], f32r, name="Y2A", tag="Y2A")
        Y2B = sp.tile([80, NY], f32r, name="Y2B", tag="Y2B")
        XPRA = sp.tile([1, NY], f32, name="XPRA", tag="XPRA")
        XPRB = sp.tile([1, NY], f32, name="XPRB", tag="XPRB")

        def stile(nm):
            return sp.tile([128, SC], f32, name=nm, tag=nm)

        SCT, ZT, Z2A, FT, GT, DT, BT, ET, HT, H2T = (
            stile(n) for n in ("SCT", "Z", "Z2A", "F", "G", "DD", "BB", "E",
                               "H", "H2"))
        D = sp.tile([64, CH], f32, name="D", tag="D")
        Z0 = sp.tile([128, 512], bf16, name="Z0", tag="Z0")

        # ---- PE warmup + clock-keeper matmuls.  The HAM clock gate only
        # credits windows where the full contraction dim streams (K=128;
        # K=10 c1 and K=64 f32 warmups never un-throttled it) AND the
        # data toggles (all-zero and constant-pattern operands never
        # registered; iota data did).  A run of K=128 bf16 matmuls on
        # iota data warms the gate during the input-DMA window, and a
        # few more dependency-free "keeper" matmuls dropped between
        # stages keep the PE busy across the evacuation-wait gaps so
        # the gate never re-throttles mid-kernel.
        for par, val in ((0, 1.0), (1, -7.3)):
            nc.vector.memset(
                bass.AP(tensor=Z0.tensor, offset=Z0.offset + par,
                        ap=[[512, 128], [2, 256]]), val)
        nc.gpsimd.memset(SCT[0:1, :], 0.0)
        nc.gpsimd.memset(SCT[64:65, :], 0.0)
        nc.gpsimd.memset(BT[:, 0:1], 0.0)
        psW = pp.tile([128, 1024], f32, name="ps", tag="ps")

        def keeper(n):
            for _ in range(n):
                nc.tensor.matmul(psW[:, 0:512], Z0[:, 0:128], Z0[:, 0:512],
                                 start=True, stop=True)

        keeper(6)

        # ================= c1 (both passes in one M=128 matmul) =========
        # 512-col groups so evacuations pipeline with the matmuls.
        def shift_copy(eng, DST, goff, gw, shift):
            if goff == 0:
                eng.tensor_copy(DST[64:128, 0:gw - shift],
                                DST[0:64, shift:gw])
            else:
                eng.tensor_copy(DST[64:128, goff - shift:goff + gw - shift],
                                DST[0:64, goff:goff + gw])

        for goff, gw in _groups(NC1, 512):
            ps = pp.tile([128, 1024], f32, name="ps", tag="ps")
            nc.tensor.matmul(ps[:, 0:gw], W1[:, :], XW[:, goff:goff + gw],
                             start=True, stop=True)
            nc.vector.tensor_scalar(SA[0:64, goff:goff + gw], ps[0:64, :gw],
                                    BIAS[0:64, 0:1], 0.0, OP.add, OP.max)
            nc.scalar.activation(SB[0:64, goff:goff + gw], ps[64:128, :gw],
                                 AF.Relu, bias=BIAS[64:128, 0:1])
            shift_copy(nc.vector, SA, goff, gw, 5)
            shift_copy(nc.vector, SB, goff, gw, 5)

        # ============ c2/c3: col-tiled A||B dilated convs (bf16) ========
        # Stacked [lo; lo<<dil] sources so each conv is 3 K=128 matmuls
        # (K=128 is also what keeps the HAM clock gate fed — K<=64 work
        # does not register and the PE clock halves).  Keepers pad the
        # per-group evacuation wait so the gate never sees an idle
        # window.
        def conv_stage(SRCA, SRCB, DSTA, DSTB, Wt, bcol, n_out, dil, shift):
            for gi, (goff, gw) in enumerate(_groups(n_out)):
                ps = pp.tile([128, 1024], f32, name="ps", tag="ps")
                for bo, nb in _subs(gw):
                    o = goff + bo
                    for t in range(3):
                        for SRC, rows in ((SRCA, slice(0, 64)),
                                          (SRCB, slice(64, 128))):
                            nc.tensor.matmul(
                                ps[rows, bo:bo + nb],
                                Wt[:, 64 * t:64 * t + 64],
                                SRC[:, o + 2 * dil * t:o + 2 * dil * t + nb],
                                start=(t == 0), stop=(t == 2),
                                skip_group_check=True)
                nc.vector.tensor_scalar(DSTA[0:64, goff:goff + gw],
                                        ps[0:64, :gw],
                                        BIAS[0:64, bcol:bcol + 1], 0.0,
                                        OP.add, OP.max)
                nc.scalar.activation(DSTB[0:64, goff:goff + gw],
                                     ps[64:128, :gw], AF.Relu,
                                     bias=BIAS[64:128, bcol:bcol + 1])
                shift_copy(nc.vector, DSTA, goff, gw, shift)
                shift_copy(nc.vector, DSTB, goff, gw, shift)

        conv_stage(SA, SB, TA, TB, W2, 1, NC2, 5, 15)
        conv_stage(TA, TB, UA, UB, W3, 2, NC3, 15, 30)

        # ======================= fc1 (bf16) =============================
        for goff, gw in NY_GROUPS:
            for c in range(2):      # M=128 chunks; F1S pair row-tiled
                psA = pp.tile([128, 1024], f32, name="ps", tag="ps")
                psB = pp.tile([128, 1024], f32, name="ps", tag="ps")
                for bo, nb in _subs(gw):
                    o = goff + bo
                    for ps_, U in ((psA, UA), (psB, UB)):
                        for p in range(3):
                            nc.tensor.matmul(
                                ps_[:, bo:bo + nb],
                                F1P[:, 320 * p + 128 * c:320 * p + 128 * c + 128],
                                U[:, o + 60 * p:o + 60 * p + nb],
                                start=(p == 0), stop=False)
                    nc.tensor.matmul(
                        psA[:, bo:bo + nb], F1SD[0:64, 128 * c:128 * c + 128],
                        UA[0:64, o + 180:o + 180 + nb],
                        start=False, stop=True)
                    nc.tensor.matmul(
                        psB[:, bo:bo + nb], F1SD[64:128, 128 * c:128 * c + 128],
                        UB[64:128, o + 150:o + 150 + nb],
                        start=False, stop=True)
                nc.vector.tensor_scalar(Y1[("A", c)][:, goff:goff + gw],
                                        psA[:, :gw], BIAS[:, 3 + c:4 + c],
                                        0.0, OP.add, OP.max)
                nc.scalar.activation(Y1[("B", c)][:, goff:goff + gw],
                                     psB[:, :gw], AF.Relu,
                                     bias=BIAS[:, 3 + c:4 + c])
            # c=2 chunk (M=64): col-tiled A||B, F1S quad-tiled
            psC = pp.tile([128, 1024], f32, name="ps", tag="ps")
            for bo, nb in _subs(gw):
                o = goff + bo
                for p in range(3):
                    for U, rows in ((UA, slice(0, 64)), (UB, slice(64, 128))):
                        nc.tensor.matmul(
                            psC[rows, bo:bo + nb],
                            F1P[:, 320 * p + 256:320 * p + 320],
                            U[:, o + 60 * p:o + 60 * p + nb],
                            start=(p == 0), stop=False,
                            skip_group_check=True)
                nc.tensor.matmul(psC[0:64, bo:bo + nb],
                                 F1SD[0:64, 256:320],
                                 UA[0:64, o + 180:o + 180 + nb],
                                 start=False, stop=True,
                                 skip_group_check=True)
                nc.tensor.matmul(psC[64:128, bo:bo + nb],
                                 F1SD[64:128, 256:320],
                                 UB[64:128, o + 150:o + 150 + nb],
                                 start=False, stop=True,
                                 skip_group_check=True)
            nc.vector.tensor_scalar(Y12[:, goff:goff + gw], psC[:, :gw],
                                    BIAS[:, 5:6], 0.0, OP.add, OP.max)

        # ======================= fc2 (bf16, Y2 only) ====================
        for goff, gw in NY_GROUPS:
            psA = pp.tile([128, 1024], f32, name="ps", tag="ps")
            psB = pp.tile([128, 1024], f32, name="ps", tag="ps")
            for bo, nb in _subs(gw):
                o = goff + bo
                for ps_, k0, k1 in ((psA, Y1[("A", 0)], Y1[("A", 1)]),
                                    (psB, Y1[("B", 0)], Y1[("B", 1)])):
                    nc.tensor.matmul(ps_[:80, bo:bo + nb], F2[:, 0:80],
                                     k0[:, o:o + nb], start=True, stop=False)
                    nc.tensor.matmul(ps_[:80, bo:bo + nb], F2[:, 80:160],
                                     k1[:, o:o + nb], start=False, stop=False)
                nc.tensor.matmul(psA[:80, bo:bo + nb], F2SD[0:64, 0:80],
                                 Y12[0:64, o:o + nb], start=False, stop=True)
                nc.tensor.matmul(psB[:80, bo:bo + nb], F2SD[64:128, 0:80],
                                 Y12[64:128, o:o + nb], start=False, stop=True)
            nc.vector.tensor_scalar(Y2A[:, goff:goff + gw], psA[:80, :gw],
                                    BIAS[0:80, 6:7], 0.0, OP.add, OP.max)
            nc.scalar.activation(Y2B[:, goff:goff + gw], psB[:80, :gw],
                                 AF.Relu, bias=BIAS[0:80, 6:7])

        # ============== VV batch (f32r, M=1, dst partition 0) ===========
        # Each NY group's xp chunk rows gather into SCT as soon as that
        # group's XPR columns land (rows r with 33(r-1)+77 <= group end),
        # so only the last small gather trails the final VV matmul.
        # SCT rows 1:64 = pass-A chunk rows 1..63, rows 65:128 = pass B.
        # Row r covers xp positions s-44+33*(r-1)+j, j in [0,77).
        # rows r with 33(r-1)+77 <= group end: 1024 -> r<=29, 1868 -> r<=55
        GATHER_ROWS = [(1, 30), (30, 56), (56, 64)]
        for gi, (goff, gw) in enumerate(NY_GROUPS):
            psA3 = pp.tile([128, 1024], f32, name="ps", tag="ps")
            psB3 = pp.tile([128, 1024], f32, name="ps", tag="ps")
            for bo, nb in _subs(gw):
                o = goff + bo
                nc.tensor.matmul(psA3[0:1, bo:bo + nb], VV[:, :],
                                 Y2A[:, o:o + nb], start=True, stop=True)
                nc.tensor.matmul(psB3[0:1, bo:bo + nb], VV[:, :],
                                 Y2B[:, o:o + nb], start=True, stop=True)
            nc.vector.tensor_scalar(XPRA[0:1, goff:goff + gw], psA3[0:1, :gw],
                                    c0, None, OP.add)
            nc.scalar.activation(XPRB[0:1, goff:goff + gw], psB3[0:1, :gw],
                                 AF.Copy, bias=c0)
            r0, r1 = GATHER_ROWS[gi]
            engs = [(nc.gpsimd, nc.scalar), (nc.gpsimd, nc.sync),
                    (nc.sync, nc.scalar)][gi]
            for XPR, rbase, eng in ((XPRA, 0, engs[0]), (XPRB, 64, engs[1])):
                src = bass.AP(tensor=XPR.tensor,
                              offset=XPR.offset + CH * (r0 - 1),
                              ap=[[NY, 1], [CH, r1 - r0], [1, SC]])
                eng.dma_start(SCT[rbase + r0:rbase + r1, :], src)
        nc.vector.tensor_tensor(SCT[:, :], SCT[:, :], MASK[:, :], OP.mult)
        nc.vector.tensor_copy(ZT[:, 0:1], SCT[:, 0:1])
        nc.scalar.activation(HT[:, :], SCT[:, :], AF.Tanh)
        cur, nxt = HT, H2T
        for it in range(SCAN_ITERS):
            if it == 0:
                nc.vector.scalar_tensor_tensor(ZT[:, 1:SC], cur[:, 0:SC - 1],
                                               a, SCT[:, 1:SC], OP.mult,
                                               OP.add)
            nc.scalar.activation(FT[:, :], ZT[:, :], AF.Tanh)
            nc.vector.tensor_tensor(GT[:, :], FT[:, :], FT[:, :], OP.mult)
            nc.vector.tensor_scalar(GT[:, :], GT[:, :], -a, a, OP.mult,
                                    OP.add)
            nc.vector.tensor_tensor(DT[:, :], FT[:, :], cur[:, :],
                                    OP.subtract)
            nc.vector.tensor_tensor(BT[:, 1:SC], GT[:, 1:SC],
                                    DT[:, 0:SC - 1], OP.mult)
            nc.vector.tensor_tensor_scan(ET[:, :], GT[:, :], BT[:, :],
                                         0.0, OP.mult, OP.add)
            if it < SCAN_ITERS - 1:
                # next Z = a*(F+E)_shift + sct; the a*F_shift + sct half
                # (Z2A) is off the critical chain, so the junction costs
                # one link after the scan instead of two (nxt, then Z).
                nc.vector.scalar_tensor_tensor(Z2A[:, 1:SC],
                                               FT[:, 0:SC - 1], a,
                                               SCT[:, 1:SC], OP.mult, OP.add)
                nc.vector.scalar_tensor_tensor(ZT[:, 1:SC], ET[:, 0:SC - 1],
                                               a, Z2A[:, 1:SC], OP.mult,
                                               OP.add)
            nc.vector.tensor_tensor(nxt[:, :], FT[:, :], ET[:, :], OP.add)
            cur, nxt = nxt, cur

        # y[p] = (hA[p] - hB[p]) / 2: binary DVE ops need equal src base
        # partitions, so bounce B's half down via a unary copy first.
        nc.vector.tensor_copy(nxt[0:64, KW:SC], cur[64:128, KW:SC])
        nc.vector.tensor_tensor(D[:, :], cur[0:64, KW:SC],
                                nxt[0:64, KW:SC], OP.subtract)
        nc.vector.tensor_scalar(D[:, :], D[:, :], 0.5, None, OP.mult)
        nc.sync.dma_start(
            y_d.ap()[0, 0:62 * CH].rearrange("(r c) -> r c", c=CH),
            D[1:63, :])
        nc.sync.dma_start(y_d.ap()[0:1, 62 * CH:P], D[63:64, 0:P - 62 * CH])

    nc.compile()
    return nc


def _prep_inputs(inputs):
    """Host-side packing: per-core input dicts."""
    import ml_dtypes
    bfd = ml_dtypes.bfloat16

    x0 = np.asarray(inputs["x0"], np.float32)[0]
    w1 = np.asarray(inputs["conv1_w"], np.float32)
    b1 = np.asarray(inputs["conv1_b"], np.float32)
    w2 = np.asarray(inputs["conv2_w"], np.float32)
    b2 = np.asarray(inputs["conv2_b"], np.float32)
    w3 = np.asarray(inputs["conv3_w"], np.float32)
    b3 = np.asarray(inputs["conv3_b"], np.float32)
    f1w = np.asarray(inputs["fc1_w"], np.float32)
    f1b = np.asarray(inputs["fc1_b"], np.float32)
    f2w = np.asarray(inputs["fc2_w"], np.float32)
    f2b = np.asarray(inputs["fc2_b"], np.float32)
    f3w = np.asarray(inputs["fc3_w"], np.float32)
    f3b = np.asarray(inputs["fc3_b"], np.float32)
    wih = np.asarray(inputs["rnn_wih"], np.float32)
    whh = np.asarray(inputs["rnn_whh"], np.float32)
    bih = np.asarray(inputs["rnn_bih"], np.float32)
    bhh = np.asarray(inputs["rnn_bhh"], np.float32)

    a = float(whh[0, 0])
    v = (wih @ f3w)[0]
    c0 = float((wih @ f3b + bih + bhh)[0])

    W1 = np.zeros((10, 128), np.float32)
    for c in range(2):
        for k in range(5):
            W1[c * 5 + k, 0:64] = w1[:, c, k]
            W1[c * 5 + k, 64:128] = w1[:, c, k] * (-1.0 if c == 0 else 1.0)

    def pack_pairs(w):  # (64,64,6) -> [128, 192]
        out = np.zeros((128, 192), np.float32)
        for t in range(3):
            out[0:64, 64 * t:64 * t + 64] = w[:, :, 2 * t].T
            out[64:128, 64 * t:64 * t + 64] = w[:, :, 2 * t + 1].T
        return out

    W2 = pack_pairs(w2)
    W3 = pack_pairs(w3)

    f1r = f1w.reshape(320, 64, 7)  # flat index = ch*7 + m
    F1P = np.zeros((128, 960), np.float32)
    for p in range(3):
        F1P[0:64, 320 * p:320 * p + 320] = f1r[:, :, 2 * p].T
        F1P[64:128, 320 * p:320 * p + 320] = f1r[:, :, 2 * p + 1].T
    F1SD = np.zeros((128, 320), np.float32)
    F1SD[0:64] = f1r[:, :, 6].T
    F1SD[64:128] = f1r[:, :, 6].T

    F2 = np.zeros((128, 160), np.float32)
    F2[:, 0:80] = f2w[:, 0:128].T
    F2[:, 80:160] = f2w[:, 128:256].T
    F2SD = np.zeros((128, 80), np.float32)
    F2SD[0:64] = f2w[:, 256:320].T
    F2SD[64:128] = f2w[:, 256:320].T

    BIAS = np.zeros((128, 7), np.float32)
    BIAS[:, 0] = np.concatenate([b1, b1])
    BIAS[:, 1] = np.concatenate([b2, b2])
    BIAS[:, 2] = np.concatenate([b3, b3])
    BIAS[:, 3] = f1b[0:128]
    BIAS[:, 4] = f1b[128:256]
    BIAS[0:64, 5] = f1b[256:320]
    BIAS[64:128, 5] = f1b[256:320]
    BIAS[0:80, 6] = f2b

    lpad = HALO
    rpad = (7 * P - HALO + NX + 8) - L
    xpad = np.zeros((2, lpad + L + max(rpad, 0)), np.float32)
    xpad[:, lpad:lpad + L] = x0

    WB = np.concatenate([F1P, F1SD, W2, W3, F2, F2SD],
                        axis=1).astype(bfd)  # [128, 1904]
    shared = dict(wb=WB, vv=v.reshape(80, 1), bias=BIAS)

    in_maps = []
    for core in range(8):
        s = P * core
        base = lpad + s - HALO
        xww = np.zeros((10, NX + 128), np.float32)
        for c in range(2):
            for k in range(5):
                xww[c * 5 + k, 0:NX] = xpad[c, base + k:base + k + NX]
        xww[:, NX:NX + 128] = W1
        # scan mask rows 0:64 = pass A chunk rows, 64:128 = pass B (same):
        # row r>=1 col j is position s - 44 + 33*(r-1) + j; zero where
        # position < 0 (core 0 only)
        m64 = np.ones((64, SC), np.float32)
        if core == 0:
            for rr in range(1, 64):
                for j in range(SC):
                    if s - HALO + CH * (rr - 1) + j < 0:
                        m64[rr, j] = 0.0
        mask = np.concatenate([m64, m64], axis=0)
        m = dict(shared)
        m["xww"] = xww
        m["mask"] = mask
        in_maps.append(m)
    return in_maps, a, c0


LAST_RESULT = None


def kernel(**inputs) -> np.ndarray:
    global LAST_RESULT
    from concourse import bass_utils

    in_maps, a, c0 = _prep_inputs(inputs)
    nc = _build_program(a, c0)
    res = bass_utils.run_bass_kernel_spmd(nc, in_maps, core_ids=list(range(8)))
    LAST_RESULT = res
    out = np.empty((1, W), np.float32)
    for core in range(8):
        out[0, P * core:P * core + P] = res.results[core]["y"][0]
    return out


# revision 68
# speedup vs baseline: 1.0092x; 1.0061x over previous
"""Trainium2 Bass kernel for nn_Net_25847113187867 (dense_cnn).

The reference slides W = 16384 stride-1 windows over x (1,2,L), runs
conv(s5)/conv(s3)/conv(s2) + 3-layer MLP + hidden-size-1 Elman RNN per
window, twice (second pass with x channel 0 negated), and returns the
antisymmetrized scan outputs (y - y_)/2.

Restructure (v2):
  * Window conv stack == dilated convs over the full sequence; fc3+RNN
    input row folded into one 80->1 vector on the host.
  * Channel-0 negation == negated conv1 weights; passes share x.
  * Sources stored stacked [src[p]; src[p+d]] across 128 partitions so
    each 64->64 dilated conv is 3 full-contraction matmuls per block.
    Stacked lower halves evacuated from PSUM (DVE/ACT split); shifted
    upper halves produced by SBUF->SBUF DVE copies.
  * conv2/conv3, fc1 and fc2 run in bf16 (fp32 PSUM accumulate,
    ~6e-3 rel err contribution vs the 2e-2 gate); conv1 and the final
    80->1 fold stay f32r.  bf16 legalizes PE-array col tiling (fp32
    modes require dst partition 0), so pass A and pass B run
    CONCURRENTLY in separate array halves wherever M<=64:
      - conv2/conv3: A -> psum[0:64] (col grp 0-1), B -> psum[64:128].
      - fc1 c=2 chunk (M=64): col-tiled pair + quad-tiled K=64 tap.
      - fc1/fc2 K=64 leftover taps (F1S/F2S): row-tiled pairs, B's
        weights+rhs at SBUF partitions 64:128 (B's rhs uses the
        shifted-copy identity U[64:128,c] = U[0:64,c+30]).
  * VV (fc3/RNN fold) matmuls batched after all fc2 groups (no PE
    head-of-line stall on the Y2 relu evacuation).
  * tanh scan: A and B chunk rows merged in one [128, 77] tile set, one
    DEER chain for both passes.
  * One f32 warmup matmul on zeros pre-warms the PE HAM clock gate
    during the input DMA window; input DMAs split across the two HW
    DGE queues (sync + scalar), small tensors packed.
  * 8 cores split outputs into 2048-position slices (overlapping input
    halos, no collectives).  All matmul free dims >= 256 so f32r
    streams at full PE rate.
"""

import numpy as np

L = 16684
W = 16384
P = 2048            # output positions per core
CH = 33             # scan chunk length (output steps per chunk row)
KW = 44             # per-chunk warmup halo steps (|whh|^44 * 0.33 ~ 3e-6)
SC = KW + CH        # 77 scan columns per chunk row
HALO = KW           # 44: left halo of xp positions per core
NY = 62 * CH + SC + 1  # 2124 xp positions per core: [s-44, s+2080)
NC3 = NY + 180      # 2304 c3 positions per core
NC2 = NC3 + 76      # 2380
NC1 = NC2 + 26      # 2406
NX = NC1 + 6        # 2412
SCAN_ITERS = 2

NY_GROUPS = [(0, 1024), (1024, 844), (1868, 256)]  # all blocks >= 256


def _groups(n, g=1024):
    out, o = [], 0
    while o < n:
        out.append((o, min(g, n - o)))
        o += g
    return out


def _subs(gw):
    s = [(0, min(512, gw))]
    if gw > 512:
        s.append((512, gw - 512))
    return s


def _build_program(a_const, c0_const):
    import concourse.bass as bass
    import concourse.mybir as mybir
    import concourse.tile as tile
    from concourse import bacc
    from contextlib import ExitStack

    dt = mybir.dt
    f32 = dt.float32
    f32r = dt.float32r
    bf16 = dt.bfloat16
    AF = mybir.ActivationFunctionType
    OP = mybir.AluOpType

    nc = bacc.Bacc("TRN2", target_bir_lowering=False, debug=False,
                   num_devices=8)

    # xww = xw | w1 (both [10,*] f32r); wb = f1p|f1sd|w2|w3|f2|f2sd (bf16)
    xww_d = nc.dram_tensor("xww", [10, NX + 128], f32r, kind="ExternalInput")
    wb_d = nc.dram_tensor("wb", [128, 1904], bf16, kind="ExternalInput")
    vv_d = nc.dram_tensor("vv", [80, 1], f32r, kind="ExternalInput")
    bias_d = nc.dram_tensor("bias", [128, 7], f32, kind="ExternalInput")
    mask_d = nc.dram_tensor("mask", [128, SC], f32, kind="ExternalInput")
    y_d = nc.dram_tensor("y", [1, P], f32, kind="ExternalOutput")

    a = float(a_const)
    c0 = float(c0_const)

    with ExitStack() as ctx:
        tc = ctx.enter_context(tile.TileContext(nc))
        wp = ctx.enter_context(tc.tile_pool(name="weights", bufs=1))
        sp = ctx.enter_context(tc.tile_pool(name="acts", bufs=1))
        pp = ctx.enter_context(tc.tile_pool(name="ps", bufs=4, space="PSUM"))

        # ---- weight loads: few big packed DMAs on the two DGE queues ----
        def load(eng, dram, shape, name, dtype):
            t = wp.tile(shape, dtype, name=name, tag=name)
            eng.dma_start(t[:], dram.ap())
            return t

        # xww split in two chunks so c1 can start on W1 + the first
        # window columns while the rest of x streams in.
        XWW = wp.tile([10, NX + 128], f32r, name="xww", tag="xww")
        nc.sync.dma_start(XWW[:, 0:1228], xww_d.ap()[:, 0:1228])
        nc.sync.dma_start(XWW[:, 1228:NX + 128], xww_d.ap()[:, 1228:NX + 128])
        BIAS = load(nc.scalar, bias_d, [128, 7], "bias", f32)
        WB = load(nc.scalar, wb_d, [128, 1904], "wb", bf16)
        MASK = load(nc.sync, mask_d, [128, SC], "mask", f32)
        VV = load(nc.scalar, vv_d, [80, 1], "vv", f32r)

        class _CV:
            """Column view into a packed weight tile."""

            def __init__(self, t, base, ncols):
                self.t, self.base, self.n = t, base, ncols

            def __getitem__(self, idx):
                r, c = idx
                lo = self.base + (c.start if c.start is not None else 0)
                hi = self.base + (c.stop if c.stop is not None else self.n)
                return self.t[r, lo:hi]

        XW = _CV(XWW, 128, NX)
        W1 = _CV(XWW, 0, 128)
        F1P = _CV(WB, 0, 960)
        F1SD = _CV(WB, 960, 320)
        W2 = _CV(WB, 1280, 192)
        W3 = _CV(WB, 1472, 192)
        F2 = _CV(WB, 1664, 160)
        F2SD = _CV(WB, 1824, 80)

        # ---- activation tiles (stacked [lo; lo-shifted] layout) ----
        SA = sp.tile([128, NC1], bf16, name="SA", tag="SA")
        SB = sp.tile([128, NC1], bf16, name="SB", tag="SB")
        TA = sp.tile([128, NC2], bf16, name="TA", tag="TA")
        TB = sp.tile([128, NC2], bf16, name="TB", tag="TB")
        UA = sp.tile([128, NC3], bf16, name="UA", tag="UA")
        UB = sp.tile([128, NC3], bf16, name="UB", tag="UB")
        Y1 = {(p, c): sp.tile([128, NY], bf16, name=f"Y1{p}{c}",
                              tag=f"Y1{p}{c}")
              for p in "AB" for c in (0, 1)}
        Y12 = sp.tile([128, NY], bf16, name="Y12", tag="Y12")
        Y2A = sp.tile([80, NY], f32r, name="Y2A", tag="Y2A")
        Y2B = sp.tile([80, NY], f32r, name="Y2B", tag="Y2B")
        XPRA = sp.tile([1, NY], f32, name="XPRA", tag="XPRA")
        XPRB = sp.tile([1, NY], f32, name="XPRB", tag="XPRB")

        def stile(nm):
            return sp.tile([128, SC], f32, name=nm, tag=nm)

        SCT, ZT, Z2A, FT, GT, DT, BT, ET, HT, H2T = (
            stile(n) for n in ("SCT", "Z", "Z2A", "F", "G", "DD", "BB", "E",
                               "H", "H2"))
        D = sp.tile([64, CH], f32, name="D", tag="D")
        Z0 = sp.tile([128, 512], bf16, name="Z0", tag="Z0")

        # ---- PE warmup + clock-keeper matmuls.  The HAM clock gate only
        # credits windows where the full contraction dim streams (K=128;
        # K=10 c1 and K=64 f32 warmups never un-throttled it) AND the
        # data toggles (all-zero and constant-pattern operands never
        # registered; iota data did).  A run of K=128 bf16 matmuls on
        # iota data warms the gate during the input-DMA window, and a
        # few more dependency-free "keeper" matmuls dropped between
        # stages keep the PE busy across the evacuation-wait gaps so
        # the gate never re-throttles mid-kernel.
        for par, val in ((0, 1.0), (1, -7.3)):
            nc.vector.memset(
                bass.AP(tensor=Z0.tensor, offset=Z0.offset + par,
                        ap=[[512, 128], [2, 256]]), val)
        nc.gpsimd.memset(SCT[0:1, :], 0.0)
        nc.gpsimd.memset(SCT[64:65, :], 0.0)
        nc.gpsimd.memset(BT[:, 0:1], 0.0)
        psW = pp.tile([128, 1024], f32, name="ps", tag="ps")

        def keeper(n):
            for _ in range(n):
                nc.tensor.matmul(psW[:, 0:512], Z0[:, 0:128], Z0[:, 0:512],
                                 start=True, stop=True)

        keeper(2)

        # ================= c1 (both passes in one M=128 matmul) =========
        # 512-col groups so evacuations pipeline with the matmuls.
        def shift_copy(eng, DST, goff, gw, shift):
            if goff == 0:
                eng.tensor_copy(DST[64:128, 0:gw - shift],
                                DST[0:64, shift:gw])
            else:
                eng.tensor_copy(DST[64:128, goff - shift:goff + gw - shift],
                                DST[0:64, goff:goff + gw])

        for goff, gw in _groups(NC1, 512):
            ps = pp.tile([128, 1024], f32, name="ps", tag="ps")
            nc.tensor.matmul(ps[:, 0:gw], W1[:, :], XW[:, goff:goff + gw],
                             start=True, stop=True)
            nc.vector.tensor_scalar(SA[0:64, goff:goff + gw], ps[0:64, :gw],
                                    BIAS[0:64, 0:1], 0.0, OP.add, OP.max)
            nc.scalar.activation(SB[0:64, goff:goff + gw], ps[64:128, :gw],
                                 AF.Relu, bias=BIAS[64:128, 0:1])
            shift_copy(nc.vector, SA, goff, gw, 5)
            shift_copy(nc.vector, SB, goff, gw, 5)

        # ============ c2/c3: col-tiled A||B dilated convs (bf16) ========
        # Stacked [lo; lo<<dil] sources so each conv is 3 K=128 matmuls
        # (K=128 is also what keeps the HAM clock gate fed — K<=64 work
        # does not register and the PE clock halves).  Keepers pad the
        # per-group evacuation wait so the gate never sees an idle
        # window.
        def conv_stage(SRCA, SRCB, DSTA, DSTB, Wt, bcol, n_out, dil, shift):
            for gi, (goff, gw) in enumerate(_groups(n_out)):
                ps = pp.tile([128, 1024], f32, name="ps", tag="ps")
                for bo, nb in _subs(gw):
                    o = goff + bo
                    for t in range(3):
                        for SRC, rows in ((SRCA, slice(0, 64)),
                                          (SRCB, slice(64, 128))):
                            nc.tensor.matmul(
                                ps[rows, bo:bo + nb],
                                Wt[:, 64 * t:64 * t + 64],
                                SRC[:, o + 2 * dil * t:o + 2 * dil * t + nb],
                                start=(t == 0), stop=(t == 2),
                                skip_group_check=True)
                nc.vector.tensor_scalar(DSTA[0:64, goff:goff + gw],
                                        ps[0:64, :gw],
                                        BIAS[0:64, bcol:bcol + 1], 0.0,
                                        OP.add, OP.max)
                nc.scalar.activation(DSTB[0:64, goff:goff + gw],
                                     ps[64:128, :gw], AF.Relu,
                                     bias=BIAS[64:128, bcol:bcol + 1])
                shift_copy(nc.vector, DSTA, goff, gw, shift)
                shift_copy(nc.vector, DSTB, goff, gw, shift)

        conv_stage(SA, SB, TA, TB, W2, 1, NC2, 5, 15)
        conv_stage(TA, TB, UA, UB, W3, 2, NC3, 15, 30)

        # ======================= fc1 (bf16) =============================
        for goff, gw in NY_GROUPS:
            for c in range(2):      # M=128 chunks; F1S pair row-tiled
                psA = pp.tile([128, 1024], f32, name="ps", tag="ps")
                psB = pp.tile([128, 1024], f32, name="ps", tag="ps")
                for bo, nb in _subs(gw):
                    o = goff + bo
                    for ps_, U in ((psA, UA), (psB, UB)):
                        for p in range(3):
                            nc.tensor.matmul(
                                ps_[:, bo:bo + nb],
                                F1P[:, 320 * p + 128 * c:320 * p + 128 * c + 128],
                                U[:, o + 60 * p:o + 60 * p + nb],
                                start=(p == 0), stop=False)
                    nc.tensor.matmul(
                        psA[:, bo:bo + nb], F1SD[0:64, 128 * c:128 * c + 128],
                        UA[0:64, o + 180:o + 180 + nb],
                        start=False, stop=True)
                    nc.tensor.matmul(
                        psB[:, bo:bo + nb], F1SD[64:128, 128 * c:128 * c + 128],
                        UB[64:128, o + 150:o + 150 + nb],
                        start=False, stop=True)
                nc.vector.tensor_scalar(Y1[("A", c)][:, goff:goff + gw],
                                        psA[:, :gw], BIAS[:, 3 + c:4 + c],
                                        0.0, OP.add, OP.max)
                nc.scalar.activation(Y1[("B", c)][:, goff:goff + gw],
                                     psB[:, :gw], AF.Relu,
                                     bias=BIAS[:, 3 + c:4 + c])
            # c=2 chunk (M=64): col-tiled A||B, F1S quad-tiled
            psC = pp.tile([128, 1024], f32, name="ps", tag="ps")
            for bo, nb in _subs(gw):
                o = goff + bo
                for p in range(3):
                    for U, rows in ((UA, slice(0, 64)), (UB, slice(64, 128))):
                        nc.tensor.matmul(
                            psC[rows, bo:bo + nb],
                            F1P[:, 320 * p + 256:320 * p + 320],
                            U[:, o + 60 * p:o + 60 * p + nb],
                            start=(p == 0), stop=False,
                            skip_group_check=True)
                nc.tensor.matmul(psC[0:64, bo:bo + nb],
                                 F1SD[0:64, 256:320],
                                 UA[0:64, o + 180:o + 180 + nb],
                                 start=False, stop=True,
                                 skip_group_check=True)
                nc.tensor.matmul(psC[64:128, bo:bo + nb],
                                 F1SD[64:128, 256:320],
                                 UB[64:128, o + 150:o + 150 + nb],
                                 start=False, stop=True,
                                 skip_group_check=True)
            nc.vector.tensor_scalar(Y12[:, goff:goff + gw], psC[:, :gw],
                                    BIAS[:, 5:6], 0.0, OP.add, OP.max)

        # ======================= fc2 (bf16, Y2 only) ====================
        for goff, gw in NY_GROUPS:
            psA = pp.tile([128, 1024], f32, name="ps", tag="ps")
            psB = pp.tile([128, 1024], f32, name="ps", tag="ps")
            for bo, nb in _subs(gw):
                o = goff + bo
                for ps_, k0, k1 in ((psA, Y1[("A", 0)], Y1[("A", 1)]),
                                    (psB, Y1[("B", 0)], Y1[("B", 1)])):
                    nc.tensor.matmul(ps_[:80, bo:bo + nb], F2[:, 0:80],
                                     k0[:, o:o + nb], start=True, stop=False)
                    nc.tensor.matmul(ps_[:80, bo:bo + nb], F2[:, 80:160],
                                     k1[:, o:o + nb], start=False, stop=False)
                nc.tensor.matmul(psA[:80, bo:bo + nb], F2SD[0:64, 0:80],
                                 Y12[0:64, o:o + nb], start=False, stop=True)
                nc.tensor.matmul(psB[:80, bo:bo + nb], F2SD[64:128, 0:80],
                                 Y12[64:128, o:o + nb], start=False, stop=True)
            nc.vector.tensor_scalar(Y2A[:, goff:goff + gw], psA[:80, :gw],
                                    BIAS[0:80, 6:7], 0.0, OP.add, OP.max)
            nc.scalar.activation(Y2B[:, goff:goff + gw], psB[:80, :gw],
                                 AF.Relu, bias=BIAS[0:80, 6:7])

        # ============== VV batch (f32r, M=1, dst partition 0) ===========
        # Each NY group's xp chunk rows gather into SCT as soon as that
        # group's XPR columns land (rows r with 33(r-1)+77 <= group end),
        # so only the last small gather trails the final VV matmul.
        # SCT rows 1:64 = pass-A chunk rows 1..63, rows 65:128 = pass B.
        # Row r covers xp positions s-44+33*(r-1)+j, j in [0,77).
        # rows r with 33(r-1)+77 <= group end: 1024 -> r<=29, 1868 -> r<=55
        GATHER_ROWS = [(1, 30), (30, 56), (56, 64)]
        for gi, (goff, gw) in enumerate(NY_GROUPS):
            psA3 = pp.tile([128, 1024], f32, name="ps", tag="ps")
            psB3 = pp.tile([128, 1024], f32, name="ps", tag="ps")
            for bo, nb in _subs(gw):
                o = goff + bo
                nc.tensor.matmul(psA3[0:1, bo:bo + nb], VV[:, :],
                                 Y2A[:, o:o + nb], start=True, stop=True)
                nc.tensor.matmul(psB3[0:1, bo:bo + nb], VV[:, :],
                                 Y2B[:, o:o + nb], start=True, stop=True)
            nc.vector.tensor_scalar(XPRA[0:1, goff:goff + gw], psA3[0:1, :gw],
                                    c0, None, OP.add)
            nc.scalar.activation(XPRB[0:1, goff:goff + gw], psB3[0:1, :gw],
                                 AF.Copy, bias=c0)
            r0, r1 = GATHER_ROWS[gi]
            engs = [(nc.gpsimd, nc.scalar), (nc.gpsimd, nc.sync),
                    (nc.sync, nc.scalar)][gi]
            for XPR, rbase, eng in ((XPRA, 0, engs[0]), (XPRB, 64, engs[1])):
                src = bass.AP(tensor=XPR.tensor,
                              offset=XPR.offset + CH * (r0 - 1),
                              ap=[[NY, 1], [CH, r1 - r0], [1, SC]])
                eng.dma_start(SCT[rbase + r0:rbase + r1, :], src)
        nc.vector.tensor_tensor(SCT[:, :], SCT[:, :], MASK[:, :], OP.mult)
        nc.vector.tensor_copy(ZT[:, 0:1], SCT[:, 0:1])
        nc.scalar.activation(HT[:, :], SCT[:, :], AF.Tanh)
        cur, nxt = HT, H2T
        for it in range(SCAN_ITERS):
            if it == 0:
                nc.vector.scalar_tensor_tensor(ZT[:, 1:SC], cur[:, 0:SC - 1],
                                               a, SCT[:, 1:SC], OP.mult,
                                               OP.add)
            nc.scalar.activation(FT[:, :], ZT[:, :], AF.Tanh)
            nc.vector.tensor_tensor(GT[:, :], FT[:, :], FT[:, :], OP.mult)
            nc.vector.tensor_scalar(GT[:, :], GT[:, :], -a, a, OP.mult,
                                    OP.add)
            nc.vector.tensor_tensor(DT[:, :], FT[:, :], cur[:, :],
                                    OP.subtract)
            nc.vector.tensor_tensor(BT[:, 1:SC], GT[:, 1:SC],
                                    DT[:, 0:SC - 1], OP.mult)
            nc.vector.tensor_tensor_scan(ET[:, :], GT[:, :], BT[:, :],
                                         0.0, OP.mult, OP.add)
            if it < SCAN_ITERS - 1:
                # next Z = a*(F+E)_shift + sct; the a*F_shift + sct half
                # (Z2A) is off the critical chain, so the junction costs
                # one link after the scan instead of two (nxt, then Z).
                nc.vector.scalar_tensor_tensor(Z2A[:, 1:SC],
                                               FT[:, 0:SC - 1], a,
                                               SCT[:, 1:SC], OP.mult, OP.add)
                nc.vector.scalar_tensor_tensor(ZT[:, 1:SC], ET[:, 0:SC - 1],
                                               a, Z2A[:, 1:SC], OP.mult,
                                               OP.add)
            nc.vector.tensor_tensor(nxt[:, :], FT[:, :], ET[:, :], OP.add)
            cur, nxt = nxt, cur

        # y[p] = (hA[p] - hB[p]) / 2: binary DVE ops need equal src base
        # partitions, so bounce B's half down via a unary copy first.
        nc.vector.tensor_copy(nxt[0:64, KW:SC], cur[64:128, KW:SC])
        nc.vector.tensor_tensor(D[:, :], cur[0:64, KW:SC],
                                nxt[0:64, KW:SC], OP.subtract)
        nc.vector.tensor_scalar(D[:, :], D[:, :], 0.5, None, OP.mult)
        nc.sync.dma_start(
            y_d.ap()[0, 0:62 * CH].rearrange("(r c) -> r c", c=CH),
            D[1:63, :])
        nc.sync.dma_start(y_d.ap()[0:1, 62 * CH:P], D[63:64, 0:P - 62 * CH])

    nc.compile()
    return nc


def _prep_inputs(inputs):
    """Host-side packing: per-core input dicts."""
    import ml_dtypes
    bfd = ml_dtypes.bfloat16

    x0 = np.asarray(inputs["x0"], np.float32)[0]
    w1 = np.asarray(inputs["conv1_w"], np.float32)
    b1 = np.asarray(inputs["conv1_b"], np.float32)
    w2 = np.asarray(inputs["conv2_w"], np.float32)
    b2 = np.asarray(inputs["conv2_b"], np.float32)
    w3 = np.asarray(inputs["conv3_w"], np.float32)
    b3 = np.asarray(inputs["conv3_b"], np.float32)
    f1w = np.asarray(inputs["fc1_w"], np.float32)
    f1b = np.asarray(inputs["fc1_b"], np.float32)
    f2w = np.asarray(inputs["fc2_w"], np.float32)
    f2b = np.asarray(inputs["fc2_b"], np.float32)
    f3w = np.asarray(inputs["fc3_w"], np.float32)
    f3b = np.asarray(inputs["fc3_b"], np.float32)
    wih = np.asarray(inputs["rnn_wih"], np.float32)
    whh = np.asarray(inputs["rnn_whh"], np.float32)
    bih = np.asarray(inputs["rnn_bih"], np.float32)
    bhh = np.asarray(inputs["rnn_bhh"], np.float32)

    a = float(whh[0, 0])
    v = (wih @ f3w)[0]
    c0 = float((wih @ f3b + bih + bhh)[0])

    W1 = np.zeros((10, 128), np.float32)
    for c in range(2):
        for k in range(5):
            W1[c * 5 + k, 0:64] = w1[:, c, k]
            W1[c * 5 + k, 64:128] = w1[:, c, k] * (-1.0 if c == 0 else 1.0)

    def pack_pairs(w):  # (64,64,6) -> [128, 192]
        out = np.zeros((128, 192), np.float32)
        for t in range(3):
            out[0:64, 64 * t:64 * t + 64] = w[:, :, 2 * t].T
            out[64:128, 64 * t:64 * t + 64] = w[:, :, 2 * t + 1].T
        return out

    W2 = pack_pairs(w2)
    W3 = pack_pairs(w3)

    f1r = f1w.reshape(320, 64, 7)  # flat index = ch*7 + m
    F1P = np.zeros((128, 960), np.float32)
    for p in range(3):
        F1P[0:64, 320 * p:320 * p + 320] = f1r[:, :, 2 * p].T
        F1P[64:128, 320 * p:320 * p + 320] = f1r[:, :, 2 * p + 1].T
    F1SD = np.zeros((128, 320), np.float32)
    F1SD[0:64] = f1r[:, :, 6].T
    F1SD[64:128] = f1r[:, :, 6].T

    F2 = np.zeros((128, 160), np.float32)
    F2[:, 0:80] = f2w[:, 0:128].T
    F2[:, 80:160] = f2w[:, 128:256].T
    F2SD = np.zeros((128, 80), np.float32)
    F2SD[0:64] = f2w[:, 256:320].T
    F2SD[64:128] = f2w[:, 256:320].T

    BIAS = np.zeros((128, 7), np.float32)
    BIAS[:, 0] = np.concatenate([b1, b1])
    BIAS[:, 1] = np.concatenate([b2, b2])
    BIAS[:, 2] = np.concatenate([b3, b3])
    BIAS[:, 3] = f1b[0:128]
    BIAS[:, 4] = f1b[128:256]
    BIAS[0:64, 5] = f1b[256:320]
    BIAS[64:128, 5] = f1b[256:320]
    BIAS[0:80, 6] = f2b

    lpad = HALO
    rpad = (7 * P - HALO + NX + 8) - L
    xpad = np.zeros((2, lpad + L + max(rpad, 0)), np.float32)
    xpad[:, lpad:lpad + L] = x0

    WB = np.concatenate([F1P, F1SD, W2, W3, F2, F2SD],
                        axis=1).astype(bfd)  # [128, 1904]
    shared = dict(wb=WB, vv=v.reshape(80, 1), bias=BIAS)

    in_maps = []
    for core in range(8):
        s = P * core
        base = lpad + s - HALO
        xww = np.zeros((10, NX + 128), np.float32)
        xww[:, 0:128] = W1
        for c in range(2):
            for k in range(5):
                xww[c * 5 + k, 128:128 + NX] = xpad[c, base + k:base + k + NX]
        # scan mask rows 0:64 = pass A chunk rows, 64:128 = pass B (same):
        # row r>=1 col j is position s - 44 + 33*(r-1) + j; zero where
        # position < 0 (core 0 only)
        m64 = np.ones((64, SC), np.float32)
        if core == 0:
            for rr in range(1, 64):
                for j in range(SC):
                    if s - HALO + CH * (rr - 1) + j < 0:
                        m64[rr, j] = 0.0
        mask = np.concatenate([m64, m64], axis=0)
        m = dict(shared)
        m["xww"] = xww
        m["mask"] = mask
        in_maps.append(m)
    return in_maps, a, c0


LAST_RESULT = None


def kernel(**inputs) -> np.ndarray:
    global LAST_RESULT
    from concourse import bass_utils

    in_maps, a, c0 = _prep_inputs(inputs)
    nc = _build_program(a, c0)
    res = bass_utils.run_bass_kernel_spmd(nc, in_maps, core_ids=list(range(8)))
    LAST_RESULT = res
    out = np.empty((1, W), np.float32)
    for core in range(8):
        out[0, P * core:P * core + P] = res.results[core]["y"][0]
    return out
